# revision 19
# baseline (speedup 1.0000x reference)
"""Trainium2 Bass kernel for an AttentionBlock (GroupNorm + single-head
self-attention + residual) over x[8, 512, 64, 64].

Sharding: data-parallel over batch — one batch element per NeuronCore (8 cores).
Per-core layout is channel-major [C=512, N=H*W=4096]; attention runs
flash-style over 512-token query blocks with scores kept transposed
[key, query] so no transposes are ever needed:

  GroupNorm is folded into the QKV weights (w*a[c]) and biases, so the
  normalized activations are never materialized.  K' [c,m] and V_tok [m,d]
  are computed once and kept in SBUF; per query block, S^T = K'^T Q' is
  accumulated in PSUM, exponentiated on the scalar engine (no max-subtraction:
  scores are ~N(0,1), exp is safe in fp32), the softmax denominator is
  accumulated on the vector engine and reduced across partitions with a
  ones-matmul, and P@V accumulates into 4 PSUM banks.  The 1/denom scale,
  output projection bias and residual are folded into the evictions.

Matmul dtype: float32r (full-rate PE, 11-bit-mantissa RNE inputs, fp32
accumulate) by default; ATTN_MM_F32R=0 switches to exact fp32 (1/4-rate).
All f32r operands are produced rounded (engine writes to f32r tiles, or
gpsimd casting DMAs) — walrus' BIR verifier requires it, and bitcast views
crash the exec unit.  The residual path keeps an unrounded fp32 copy of x.
"""

import os

import numpy as np

import concourse.bass as bass
import concourse.mybir as mybir
import concourse.tile as tile

from concourse.bass_utils import run_bass_kernel_spmd
from concourse.vector_clock import ScopedClock

AF = mybir.ActivationFunctionType
ALU = mybir.AluOpType
FP32 = mybir.dt.float32
F32R = mybir.dt.float32r

B = 8
C = 512
N = 4096          # H*W
G = 8             # groups
EPS = 1e-5
CT = C // 128     # 4 channel tiles
NBS = 512         # query-block size
NB = N // NBS     # 8 query blocks
MC = N // 128     # 32 key chunks
SCALE = 1.0 / np.sqrt(np.float32(C))

MM_F32R = os.environ.get("ATTN_MM_F32R", "1") == "1"
DEBUG_DUMP = os.environ.get("ATTN_DEBUG_DUMP", "0") == "1"


class _TileContext(tile.TileContext):
    """This container's walrus rejects >1 sync wait on a CTRL instruction
    ("Too many sync wait commands"); split the tail drain's waits across
    multiple drain instructions.  It also rejects long semaphore-range-clear
    ISA instructions ("ISA wrong length"); clear in chunks of <=3."""

    def _drain_and_barrier(self, tick_clock, wait_clock):
        drain_inst = self.nc.sync.drain()
        wait_clock.add_sem_waits(
            drain_inst.ins, ScopedClock({None: tick_clock.global_clock})
        )
        si = drain_inst.ins.sync_info
        if si is not None and si.on_wait and len(si.on_wait) > 1:
            waits = list(si.on_wait)
            drain_inst.ins.sync_info = mybir.SyncInfo(
                on_wait=[waits[0]], on_update=list(si.on_update)
            )
            for w in waits[1:]:
                d = self.nc.sync.drain()
                d.ins.sync_info = mybir.SyncInfo(on_wait=[w], on_update=[])

        self.nc.all_engine_barrier()
        assert self.sems is not None
        popped = self.nc._tile_sem_poison_stack.pop()
        assert popped is self._sem_poison
        sems = list(self.sems.allocated().values())
        for i in range(0, len(sems), 3):
            self.nc.clear_and_free_semaphores(sems[i:i + 3])
        self.nc.all_engine_barrier()


def _split_multi_waits(nc, limit=1):
    """This container's walrus accepts at most one sync wait per instruction.
    Hoist extra waits onto same-engine EventSemaphore instructions inserted
    just before — equivalent ordering (engines execute in program order)."""
    nid = 0
    for f in nc.m.functions:
        for bb in f.blocks:
            out = []
            changed = False
            for inst in bb.instructions:
                si = inst.sync_info
                if si is not None and si.on_wait and len(si.on_wait) > limit:
                    waits = list(si.on_wait)
                    for w in waits[:-limit]:
                        ev = mybir.InstEventSemaphore(
                            name=f"I-wsplit-{nid}",
                            engine=inst.engine,
                            sync_info=mybir.SyncInfo(on_wait=[w], on_update=[]),
                        )
                        nid += 1
                        out.append(ev)
                    inst.sync_info = mybir.SyncInfo(
                        on_wait=waits[-limit:], on_update=list(si.on_update)
                    )
                    changed = True
                out.append(inst)
            if changed:
                bb.instructions = out


def _build_kernel():
    DT = F32R if MM_F32R else FP32
    nc = bass.Bass()

    x = nc.declare_dram_parameter("x", [C, N], FP32, isOutput=False)
    wqT = nc.declare_dram_parameter("wqT", [C, C], FP32, isOutput=False)
    wkT = nc.declare_dram_parameter("wkT", [C, C], FP32, isOutput=False)
    wvT = nc.declare_dram_parameter("wvT", [C, C], FP32, isOutput=False)
    woT = nc.declare_dram_parameter("woT", [C, C], FP32, isOutput=False)
    gnw = nc.declare_dram_parameter("gnw", [C], FP32, isOutput=False)
    gnb = nc.declare_dram_parameter("gnb", [C], FP32, isOutput=False)
    bq = nc.declare_dram_parameter("bq", [C], FP32, isOutput=False)
    bk = nc.declare_dram_parameter("bk", [C], FP32, isOutput=False)
    bv = nc.declare_dram_parameter("bv", [C], FP32, isOutput=False)
    bo = nc.declare_dram_parameter("bo", [C], FP32, isOutput=False)
    # group-indicator constants for the cross-partition GroupNorm reductions
    ind128 = nc.declare_dram_parameter("ind128", [128, 2], FP32, isOutput=False)
    indT2 = nc.declare_dram_parameter("indT2", [128, 128], FP32, isOutput=False)
    y = nc.declare_dram_parameter("y", [C, N], FP32, isOutput=True)
    dbg = {}
    if DEBUG_DUMP:
        for nm, shp in [
            ("dbg_stats", [128, 8]), ("dbg_a", [128, CT]),
            ("dbg_qb", [128, CT]), ("dbg_bo", [128, CT]),
            ("dbg_q", [128, 512]), ("dbg_k", [128, 512]),
            ("dbg_v", [128, 512]), ("dbg_p", [128, 512]),
            ("dbg_dn", [128, 512]), ("dbg_rb", [128, 512]),
        ]:
            dbg[nm] = nc.declare_dram_parameter(nm, shp, FP32, isOutput=True)

    x_r = x[:].rearrange("(t p) m -> t p m", p=128)   # [4, 128, 4096]
    y_r = y[:].rearrange("(t p) m -> t p m", p=128)

    def dma_cast(out, in_):
        # only gpsimd DMAs may cast fp32 -> f32r
        if out.dtype != in_.dtype:
            nc.gpsimd.dma_start(out=out, in_=in_)
        else:
            nc.sync.dma_start(out=out, in_=in_)

    with _TileContext(nc) as tc:
        with (
            tc.tile_pool(name="small", bufs=1) as small,
            tc.tile_pool(name="wmm", bufs=1) as wmm,
            tc.tile_pool(name="ps_mm", bufs=2, space="PSUM") as ps_mm,
        ):
            # ---- persistent: folded/rounded weights used in phase 4 ----
            wq_mm = wmm.tile([128, CT, C], DT, tag="wqm")
            wo_mm = wmm.tile([128, CT, C], DT, tag="wom")

            ind128_sb = small.tile([128, 2], FP32, tag="ind128")
            indT2_sb = small.tile([128, 128], FP32, tag="indT2")
            nc.sync.dma_start(out=ind128_sb, in_=ind128[:])
            nc.sync.dma_start(out=indT2_sb, in_=indT2[:])

            def load_pc(name, dram):  # [512] -> [128, 4] (channel = t*128+p)
                t = small.tile([128, CT], FP32, tag=name)
                nc.sync.dma_start(out=t, in_=dram[:].rearrange("(t p) -> p t", p=128))
                return t

            gnw_sb = load_pc("gnw", gnw)
            gnb_sb = load_pc("gnb", gnb)
            bq_sb = load_pc("bq", bq)
            bk_sb = load_pc("bk", bk)
            bv_sb = load_pc("bv", bv)
            bo_sb = load_pc("bo", bo)

            eps_sb = small.tile([128, 1], FP32, tag="eps")
            nc.vector.memset(eps_sb, EPS)
            ones128_sb = small.tile([128, 1], FP32, tag="ones128")
            nc.vector.memset(ones128_sb, 1.0)
            ones1_sb = small.tile([128, 128], FP32, tag="ones1")
            nc.vector.memset(ones1_sb, 1.0)

            pcs = small.tile([128, 8], FP32, tag="pcs")        # (s,t): s*4+t
            stats128 = small.tile([128, 8], FP32, tag="st128")  # (j,t): j*4+t
            a_pc = small.tile([128, CT], FP32, tag="a_pc")
            beff = small.tile([128, CT], FP32, tag="beff")
            qbias = small.tile([128, CT], FP32, tag="qbias")
            kbias = small.tile([128, CT], FP32, tag="kbias")
            vbias = small.tile([128, CT], FP32, tag="vbias")
            boeff = small.tile([128, CT], FP32, tag="boeff")

            with tc.tile_pool(name="wkvmm", bufs=1) as wkvmm:
                wk_mm = wkvmm.tile([128, CT, C], DT, tag="wkm")
                wv_mm = wkvmm.tile([128, CT, C], DT, tag="wvm")

                with tc.tile_pool(name="wraw", bufs=1) as wraw:
                    wq_sb = wraw.tile([128, CT, C], FP32, tag="wq")
                    wk_sb = wraw.tile([128, CT, C], FP32, tag="wk")
                    wv_sb = wraw.tile([128, CT, C], FP32, tag="wv")
                    wo_sb = wraw.tile([128, CT, C], FP32, tag="wo")

                    # ============ phase 1: GroupNorm statistics =============
                    with (
                        tc.tile_pool(name="xstat", bufs=2) as xstat,
                        tc.tile_pool(name="sttmp", bufs=4) as sttmp,
                    ):
                        for ct in range(CT):
                            xt = xstat.tile([128, N], FP32, tag="xt")
                            nc.sync.dma_start(out=xt, in_=x_r[ct])
                            st = sttmp.tile([128, 8, 6], FP32, tag="st")
                            for j in range(8):
                                nc.vector.bn_stats(
                                    out=st[:, j], in_=xt[:, j * 512:(j + 1) * 512]
                                )
                            mv = sttmp.tile([128, 2], FP32, tag="mv")
                            nc.vector.bn_aggr(out=mv, in_=st)
                            # pcs[:, ct]=mean ; pcs[:, 4+ct]=E[x^2]=var+mean^2
                            nc.vector.tensor_copy(pcs[:, ct:ct + 1], mv[:, 0:1])
                            m2 = sttmp.tile([128, 1], FP32, tag="m2")
                            nc.vector.tensor_mul(m2, mv[:, 0:1], mv[:, 0:1])
                            nc.vector.tensor_add(
                                pcs[:, 4 + ct:5 + ct], mv[:, 1:2], m2
                            )

                    # weight loads after the stats x-loads: stats are
                    # the serial head, weights only gate phase 2
                    for t, d in ((wk_sb, wkT), (wq_sb, wqT),
                                 (wv_sb, wvT), (wo_sb, woT)):
                        nc.sync.dma_start(
                            out=t, in_=d[:].rearrange("(t p) d -> p t d", p=128)
                        )

                    # group sums over the 64 member channels' stats
                    gs_ps = ps_mm.tile([128, 512], FP32, tag="mm")
                    nc.tensor.matmul(
                        gs_ps[:2, :8], lhsT=ind128_sb, rhs=pcs, start=True, stop=True
                    )
                    gs_sb = small.tile([128, 8], FP32, tag="gs")
                    nc.scalar.activation(
                        gs_sb[:2], gs_ps[:2, :8], AF.Copy, scale=1.0 / (C // G)
                    )
                    nc.vector.memset(stats128, 0.0)
                    nc.vector.tensor_copy(stats128[:2, 0:4], gs_sb[:2, 0:4])
                    vtmp = small.tile([128, 4], FP32, tag="vtmp")
                    nc.vector.tensor_mul(vtmp[:2], gs_sb[:2, 0:4], gs_sb[:2, 0:4])
                    nc.vector.tensor_sub(
                        stats128[:2, 4:8], gs_sb[:2, 4:8], vtmp[:2]
                    )
                    nc.scalar.activation(
                        stats128[:2, 4:8], stats128[:2, 4:8], AF.Sqrt,
                        bias=eps_sb[:2],
                    )
                    nc.vector.reciprocal(stats128[:2, 4:8], stats128[:2, 4:8])

                    # broadcast group stats back to channels: bc[p, (j,t)]
                    bc_ps = ps_mm.tile([128, 512], FP32, tag="mm")
                    nc.tensor.matmul(
                        bc_ps[:, :8], lhsT=indT2_sb, rhs=stats128,
                        start=True, stop=True,
                    )
                    bc_sb = small.tile([128, 8], FP32, tag="bc")
                    nc.scalar.copy(bc_sb, bc_ps[:, :8])
                    # a = rstd * gn_w ; beff = gn_b - mean * a
                    nc.vector.tensor_mul(a_pc, bc_sb[:, 4:8], gnw_sb)
                    nc.vector.tensor_mul(beff, bc_sb[:, 0:4], a_pc)
                    nc.vector.tensor_sub(beff, gnb_sb, beff)

                    # ====== phase 2: fold GN into biases and weights ========
                    # qbias[d] = bq[d] + sum_c wqT[c, d] * beff[c]   (etc.)
                    for w_sb, b_sb, out_t in (
                        (wq_sb, bq_sb, qbias),
                        (wk_sb, bk_sb, kbias),
                        (wv_sb, bv_sb, vbias),
                    ):
                        b_ps = ps_mm.tile([128, 512], FP32, tag="mm")
                        for dt in range(CT):
                            for ct in range(CT):
                                nc.tensor.matmul(
                                    b_ps[:, dt:dt + 1],
                                    lhsT=w_sb[:, ct, dt * 128:(dt + 1) * 128],
                                    rhs=beff[:, ct:ct + 1],
                                    start=(ct == 0),
                                    stop=(ct == CT - 1),
                                )
                        nc.vector.tensor_add(out_t, b_ps[:, 0:CT], b_sb)

                    # boeff[e] = bo[e] + sum_d woT[d, e] * vbias[d]
                    bo_ps = ps_mm.tile([128, 512], FP32, tag="mm")
                    for et in range(CT):
                        for dt in range(CT):
                            nc.tensor.matmul(
                                bo_ps[:, et:et + 1],
                                lhsT=wo_sb[:, dt, et * 128:(et + 1) * 128],
                                rhs=vbias[:, dt:dt + 1],
                                start=(dt == 0),
                                stop=(dt == CT - 1),
                            )
                    nc.vector.tensor_add(boeff, bo_ps[:, 0:CT], bo_sb)

                    if DEBUG_DUMP:
                        nc.sync.dma_start(out=dbg["dbg_stats"][:], in_=stats128)
                        nc.sync.dma_start(out=dbg["dbg_a"][:], in_=a_pc)
                        nc.sync.dma_start(out=dbg["dbg_qb"][:], in_=qbias)
                        nc.sync.dma_start(out=dbg["dbg_bo"][:], in_=boeff)

                    # fold a[c] into wq/wk/wv rows (rounding to DT on write);
                    # wo is just rounded
                    for w_sb, w_m in (
                        (wk_sb, wk_mm), (wq_sb, wq_mm), (wv_sb, wv_mm)
                    ):
                        for ct in range(CT):
                            nc.vector.tensor_scalar_mul(
                                w_m[:, ct, :], w_sb[:, ct, :], a_pc[:, ct:ct + 1]
                            )
                    nc.vector.tensor_copy(wo_mm, wo_sb)

                # ========== phase 3: K' [c, m] and V_tok [m, d] =============
                with tc.tile_pool(name="kv", bufs=1) as kvp:
                    k_full = kvp.tile([128, CT, N], DT, tag="k_full")
                    v_full = kvp.tile([128, MC, 512], DT, tag="v_full")

                    with (
                        tc.tile_pool(name="xq", bufs=4) as xq,
                        tc.tile_pool(name="qp", bufs=4) as qpool,
                    ):
                        def emit_qproj(nb):
                            """x cast-load + Q' projection for block nb;
                            emitted one block ahead so the matmuls fill the
                            PE while the denom chain of the previous block
                            runs on DVE/ACT."""
                            nsl_q = slice(nb * NBS, (nb + 1) * NBS)
                            xqs = []
                            for ct in range(CT):
                                xtq = xq.tile([128, NBS], DT, tag="xq",
                                              name=f"xq{nb}_{ct}")
                                dma_cast(xtq, x_r[ct][:, nsl_q])
                                xqs.append(xtq)
                            qs = []
                            for dt in range(CT):
                                qp_ps = ps_mm.tile([128, 512], FP32, tag="mm",
                                                   name=f"qps{nb}_{dt}")
                                for ct in range(CT):
                                    nc.tensor.matmul(
                                        qp_ps,
                                        lhsT=wq_mm[:, ct, dt * 128:(dt + 1) * 128],
                                        rhs=xqs[ct],
                                        start=(ct == 0),
                                        stop=(ct == CT - 1),
                                    )
                                qt = qpool.tile([128, NBS], DT, tag="q",
                                                name=f"q{nb}_{dt}")
                                nc.vector.tensor_scalar_add(
                                    qt, qp_ps, qbias[:, dt:dt + 1]
                                )
                                qs.append(qt)
                            return qs

                        qs_cur = emit_qproj(0)

                        with tc.tile_pool(name="xmc", bufs=6) as xmc:
                            for m2 in range(8):
                                sl = slice(m2 * 512, (m2 + 1) * 512)
                                xts = []
                                for ct in range(CT):
                                    xt = xmc.tile([128, 512], DT, tag="xmc")
                                    dma_cast(xt, x_r[ct][:, sl])
                                    xts.append(xt)
                                for dt in range(CT):
                                    kp = ps_mm.tile([128, 512], FP32, tag="mm")
                                    for ct in range(CT):
                                        nc.tensor.matmul(
                                            kp,
                                            lhsT=wk_mm[:, ct, dt * 128:(dt + 1) * 128],
                                            rhs=xts[ct],
                                            start=(ct == 0),
                                            stop=(ct == CT - 1),
                                        )
                                    nc.vector.tensor_scalar_add(
                                        k_full[:, dt, sl], kp, kbias[:, dt:dt + 1]
                                    )
                                for mt in range(4):
                                    vp = ps_mm.tile([128, 512], FP32, tag="mm")
                                    for ct in range(CT):
                                        nc.tensor.matmul(
                                            vp,
                                            lhsT=xts[ct][:, mt * 128:(mt + 1) * 128],
                                            rhs=wv_mm[:, ct, :],
                                            start=(ct == 0),
                                            stop=(ct == CT - 1),
                                        )
                                    nc.scalar.copy(v_full[:, m2 * 4 + mt, :], vp)

                        if DEBUG_DUMP:
                            nc.sync.dma_start(out=dbg["dbg_k"][:], in_=k_full[:, 0, 0:512])
                            nc.sync.dma_start(out=dbg["dbg_v"][:], in_=v_full[:, 0, :])

                        # ========== phase 4: attention per query block ======
                        with (
                            tc.tile_pool(name="xres", bufs=4) as xres,
                            tc.tile_pool(name="pp", bufs=2) as ppool,
                            tc.tile_pool(name="dn", bufs=2) as dnpool,
                            tc.tile_pool(name="op", bufs=4) as opool,
                            tc.tile_pool(name="yp", bufs=2) as ypool,
                            tc.tile_pool(name="ps_S", bufs=2, space="PSUM") as ps_s,
                            tc.tile_pool(name="ps_O", bufs=4, space="PSUM") as ps_o,
                        ):
                            for nb in range(NB):
                                nsl = slice(nb * NBS, (nb + 1) * NBS)
                                xrs = []
                                for ct in range(CT):
                                    xtr = xres.tile([128, NBS], FP32, tag="xres")
                                    nc.sync.dma_start(out=xtr, in_=x_r[ct][:, nsl])
                                    xrs.append(xtr)
                                qs = qs_cur

                                dn = dnpool.tile([128, NBS], FP32, tag="dn")
                                nc.vector.memset(dn, 0.0)
                                o_ps = [
                                    ps_o.tile([128, 512], FP32, tag="o",
                                              name=f"o_ps{dt}")
                                    for dt in range(CT)
                                ]
                                for mc in range(MC):
                                    sp = ps_s.tile([128, 512], FP32, tag="s")
                                    for dt in range(CT):
                                        nc.tensor.matmul(
                                            sp,
                                            lhsT=k_full[:, dt, mc * 128:(mc + 1) * 128],
                                            rhs=qs[dt],
                                            start=(dt == 0),
                                            stop=(dt == CT - 1),
                                        )
                                    pb = ppool.tile([128, NBS], DT, tag="p")
                                    nc.scalar.activation(
                                        pb, sp, AF.Exp, scale=float(SCALE)
                                    )
                                    if DEBUG_DUMP and nb == 0 and mc == 0:
                                        nc.sync.dma_start(out=dbg["dbg_p"][:], in_=pb)
                                    nc.vector.tensor_add(dn, dn, pb)
                                    for dt in range(CT):
                                        nc.tensor.matmul(
                                            o_ps[dt],
                                            lhsT=v_full[:, mc, dt * 128:(dt + 1) * 128],
                                            rhs=pb,
                                            start=(mc == 0),
                                            stop=(mc == MC - 1),
                                        )

                                # O evictions (unscaled) go to ACT right away
                                os_ = []
                                for dt in range(CT):
                                    ot = opool.tile([128, NBS], DT, tag="ot")
                                    nc.scalar.copy(ot, o_ps[dt])
                                    os_.append(ot)

                                # next block's Q fills the PE while the denom
                                # chain completes on DVE/ACT
                                qs_cur = emit_qproj(nb + 1) if nb + 1 < NB else None

                                # denom = sum_m P via ones-matmul; reciprocal;
                                # broadcast back via a K=1 matmul
                                dn_ps = ps_mm.tile([128, 512], FP32, tag="mm")
                                nc.tensor.matmul(
                                    dn_ps[:1, :], lhsT=ones128_sb, rhs=dn,
                                    start=True, stop=True,
                                )
                                r1 = dnpool.tile([128, NBS], FP32, tag="dn",
                                                 name="r1")
                                nc.vector.reciprocal(r1[:1], dn_ps[:1, :])
                                rb_ps = ps_mm.tile([128, 512], FP32, tag="mm")
                                nc.tensor.matmul(
                                    rb_ps, lhsT=ones1_sb[:1], rhs=r1[:1],
                                    start=True, stop=True,
                                )
                                rb = dnpool.tile([128, NBS], FP32, tag="dn",
                                                 name="rb")
                                nc.scalar.copy(rb, rb_ps)
                                if DEBUG_DUMP and nb == 0:
                                    nc.sync.dma_start(out=dbg["dbg_dn"][:], in_=dn)
                                    nc.sync.dma_start(out=dbg["dbg_rb"][:], in_=rb)

                                for et in range(CT):
                                    op_ps = ps_mm.tile([128, 512], FP32, tag="mm")
                                    for dt in range(CT):
                                        nc.tensor.matmul(
                                            op_ps,
                                            lhsT=wo_mm[:, dt, et * 128:(et + 1) * 128],
                                            rhs=os_[dt],
                                            start=(dt == 0),
                                            stop=(dt == CT - 1),
                                        )
                                    yt = ypool.tile([128, NBS], FP32, tag="y")
                                    # y = OP*rb + boeff + x
                                    nc.vector.tensor_tensor(
                                        yt, op_ps, rb, op=ALU.mult
                                    )
                                    nc.vector.scalar_tensor_tensor(
                                        yt,
                                        yt,
                                        boeff[:, et:et + 1],
                                        xrs[et],
                                        op0=ALU.add,
                                        op1=ALU.add,
                                    )
                                    nc.sync.dma_start(out=y_r[et][:, nsl], in_=yt)
    if os.environ.get("ATTN_NO_SPLIT", "0") != "1":
        _split_multi_waits(nc)
    return nc


_NC_CACHE = {}


def _get_nc():
    key = (MM_F32R, DEBUG_DUMP)
    if key not in _NC_CACHE:
        _NC_CACHE[key] = _build_kernel()
    return _NC_CACHE[key]


def _make_in_maps(x, gn_w, gn_b, wq, bq, wk, bk, wv, bv, wo, bo):
    x = np.asarray(x, np.float32).reshape(B, C, N)
    shared = {
        "wqT": np.ascontiguousarray(np.asarray(wq, np.float32).T),
        "wkT": np.ascontiguousarray(np.asarray(wk, np.float32).T),
        "wvT": np.ascontiguousarray(np.asarray(wv, np.float32).T),
        "woT": np.ascontiguousarray(np.asarray(wo, np.float32).T),
        "gnw": np.asarray(gn_w, np.float32),
        "gnb": np.asarray(gn_b, np.float32),
        "bq": np.asarray(bq, np.float32),
        "bk": np.asarray(bk, np.float32),
        "bv": np.asarray(bv, np.float32),
        "bo": np.asarray(bo, np.float32),
    }
    ind128 = np.zeros((128, 2), np.float32)
    ind128[:64, 0] = 1.0
    ind128[64:, 1] = 1.0
    indT2 = np.zeros((128, 128), np.float32)
    indT2[0, :64] = 1.0
    indT2[1, 64:] = 1.0
    shared["ind128"] = ind128
    shared["indT2"] = indT2
    return [
        {"x": np.ascontiguousarray(x[b]), **shared} for b in range(B)
    ]


def run(inputs, trace=False, tmpdir=None):
    nc = _get_nc()
    in_maps = _make_in_maps(**inputs)
    res = run_bass_kernel_spmd(
        nc, in_maps, core_ids=list(range(B)), trace=trace, tmpdir=tmpdir
    )
    out = np.stack([res.results[b]["y"] for b in range(B)])
    return out.reshape(B, C, 64, 64).astype(np.float32), res


def kernel(**inputs):
    out, _ = run(inputs)
    return out


# revision 21
# speedup vs baseline: 1.0486x; 1.0486x over previous
"""Trainium2 Bass kernel for an AttentionBlock (GroupNorm + single-head
self-attention + residual) over x[8, 512, 64, 64].

Sharding: data-parallel over batch — one batch element per NeuronCore (8 cores).
Per-core layout is channel-major [C=512, N=H*W=4096]; attention runs
flash-style over 512-token query blocks with scores kept transposed
[key, query] so no transposes are ever needed:

  GroupNorm is folded into the QKV weights (w*a[c]) and biases, so the
  normalized activations are never materialized.  K' [c,m] and V_tok [m,d]
  are computed once and kept in SBUF; per query block, S^T = K'^T Q' is
  accumulated in PSUM, exponentiated on the scalar engine (no max-subtraction:
  scores are ~N(0,1), exp is safe in fp32), the softmax denominator is
  accumulated on the vector engine and reduced across partitions with a
  ones-matmul, and P@V accumulates into 4 PSUM banks.  The 1/denom scale,
  output projection bias and residual are folded into the evictions.

Matmul dtype: float32r (full-rate PE, 11-bit-mantissa RNE inputs, fp32
accumulate) by default; ATTN_MM_F32R=0 switches to exact fp32 (1/4-rate).
All f32r operands are produced rounded (engine writes to f32r tiles, or
gpsimd casting DMAs) — walrus' BIR verifier requires it, and bitcast views
crash the exec unit.  The residual path keeps an unrounded fp32 copy of x.
"""

import os

import numpy as np

import concourse.bass as bass
import concourse.mybir as mybir
import concourse.tile as tile

from concourse.bass_utils import run_bass_kernel_spmd
from concourse.vector_clock import ScopedClock

AF = mybir.ActivationFunctionType
ALU = mybir.AluOpType
FP32 = mybir.dt.float32
F32R = mybir.dt.float32r

B = 8
C = 512
N = 4096          # H*W
G = 8             # groups
EPS = 1e-5
CT = C // 128     # 4 channel tiles
NBS = 512         # query-block size
NB = N // NBS     # 8 query blocks
MC = N // 128     # 32 key chunks
SCALE = 1.0 / np.sqrt(np.float32(C))

MM_F32R = os.environ.get("ATTN_MM_F32R", "1") == "1"
DEBUG_DUMP = os.environ.get("ATTN_DEBUG_DUMP", "0") == "1"


class _TileContext(tile.TileContext):
    """This container's walrus rejects >1 sync wait on a CTRL instruction
    ("Too many sync wait commands"); split the tail drain's waits across
    multiple drain instructions.  It also rejects long semaphore-range-clear
    ISA instructions ("ISA wrong length"); clear in chunks of <=3."""

    def _drain_and_barrier(self, tick_clock, wait_clock):
        drain_inst = self.nc.sync.drain()
        wait_clock.add_sem_waits(
            drain_inst.ins, ScopedClock({None: tick_clock.global_clock})
        )
        si = drain_inst.ins.sync_info
        if si is not None and si.on_wait and len(si.on_wait) > 1:
            waits = list(si.on_wait)
            drain_inst.ins.sync_info = mybir.SyncInfo(
                on_wait=[waits[0]], on_update=list(si.on_update)
            )
            for w in waits[1:]:
                d = self.nc.sync.drain()
                d.ins.sync_info = mybir.SyncInfo(on_wait=[w], on_update=[])

        self.nc.all_engine_barrier()
        assert self.sems is not None
        popped = self.nc._tile_sem_poison_stack.pop()
        assert popped is self._sem_poison
        sems = list(self.sems.allocated().values())
        for i in range(0, len(sems), 3):
            self.nc.clear_and_free_semaphores(sems[i:i + 3])
        self.nc.all_engine_barrier()


def _split_multi_waits(nc, limit=1):
    """This container's walrus accepts at most one sync wait per instruction.
    Hoist extra waits onto same-engine EventSemaphore instructions inserted
    just before — equivalent ordering (engines execute in program order)."""
    nid = 0
    for f in nc.m.functions:
        for bb in f.blocks:
            out = []
            changed = False
            for inst in bb.instructions:
                si = inst.sync_info
                if si is not None and si.on_wait and len(si.on_wait) > limit:
                    waits = list(si.on_wait)
                    for w in waits[:-limit]:
                        ev = mybir.InstEventSemaphore(
                            name=f"I-wsplit-{nid}",
                            engine=inst.engine,
                            sync_info=mybir.SyncInfo(on_wait=[w], on_update=[]),
                        )
                        nid += 1
                        out.append(ev)
                    inst.sync_info = mybir.SyncInfo(
                        on_wait=waits[-limit:], on_update=list(si.on_update)
                    )
                    changed = True
                out.append(inst)
            if changed:
                bb.instructions = out


def _build_kernel():
    DT = F32R if MM_F32R else FP32
    nc = bass.Bass()

    x = nc.declare_dram_parameter("x", [C, N], FP32, isOutput=False)
    wqT = nc.declare_dram_parameter("wqT", [C, C], FP32, isOutput=False)
    wkT = nc.declare_dram_parameter("wkT", [C, C], FP32, isOutput=False)
    wvT = nc.declare_dram_parameter("wvT", [C, C], FP32, isOutput=False)
    woT = nc.declare_dram_parameter("woT", [C, C], FP32, isOutput=False)
    gnw = nc.declare_dram_parameter("gnw", [C], FP32, isOutput=False)
    gnb = nc.declare_dram_parameter("gnb", [C], FP32, isOutput=False)
    bq = nc.declare_dram_parameter("bq", [C], FP32, isOutput=False)
    bk = nc.declare_dram_parameter("bk", [C], FP32, isOutput=False)
    bv = nc.declare_dram_parameter("bv", [C], FP32, isOutput=False)
    bo = nc.declare_dram_parameter("bo", [C], FP32, isOutput=False)
    # group-indicator constants for the cross-partition GroupNorm reductions
    ind128 = nc.declare_dram_parameter("ind128", [128, 2], FP32, isOutput=False)
    indT2 = nc.declare_dram_parameter("indT2", [128, 128], FP32, isOutput=False)
    y = nc.declare_dram_parameter("y", [C, N], FP32, isOutput=True)
    dbg = {}
    if DEBUG_DUMP:
        for nm, shp in [
            ("dbg_stats", [128, 8]), ("dbg_a", [128, CT]),
            ("dbg_qb", [128, CT]), ("dbg_bo", [128, CT]),
            ("dbg_q", [128, 512]), ("dbg_k", [128, 512]),
            ("dbg_v", [128, 512]), ("dbg_p", [128, 512]),
            ("dbg_dn", [128, 512]), ("dbg_rb", [128, 512]),
        ]:
            dbg[nm] = nc.declare_dram_parameter(nm, shp, FP32, isOutput=True)

    x_r = x[:].rearrange("(t p) m -> t p m", p=128)   # [4, 128, 4096]
    y_r = y[:].rearrange("(t p) m -> t p m", p=128)

    def dma_cast(out, in_):
        # only gpsimd DMAs may cast fp32 -> f32r
        if out.dtype != in_.dtype:
            nc.gpsimd.dma_start(out=out, in_=in_)
        else:
            nc.sync.dma_start(out=out, in_=in_)

    with _TileContext(nc) as tc:
        with (
            tc.tile_pool(name="small", bufs=1) as small,
            tc.tile_pool(name="wmm", bufs=1) as wmm,
            tc.tile_pool(name="ps_mm", bufs=2, space="PSUM") as ps_mm,
        ):
            # ---- persistent: folded/rounded weights used in phase 4 ----
            wq_mm = wmm.tile([128, CT, C], DT, tag="wqm")
            wo_mm = wmm.tile([128, CT, C], DT, tag="wom")

            ind128_sb = small.tile([128, 2], FP32, tag="ind128")
            indT2_sb = small.tile([128, 128], FP32, tag="indT2")
            nc.sync.dma_start(out=ind128_sb, in_=ind128[:])
            nc.sync.dma_start(out=indT2_sb, in_=indT2[:])

            def load_pc(name, dram):  # [512] -> [128, 4] (channel = t*128+p)
                t = small.tile([128, CT], FP32, tag=name)
                nc.sync.dma_start(out=t, in_=dram[:].rearrange("(t p) -> p t", p=128))
                return t

            gnw_sb = load_pc("gnw", gnw)
            gnb_sb = load_pc("gnb", gnb)
            bq_sb = load_pc("bq", bq)
            bk_sb = load_pc("bk", bk)
            bv_sb = load_pc("bv", bv)
            bo_sb = load_pc("bo", bo)

            eps_sb = small.tile([128, 1], FP32, tag="eps")
            nc.vector.memset(eps_sb, EPS)
            ones128_sb = small.tile([128, 1], FP32, tag="ones128")
            nc.vector.memset(ones128_sb, 1.0)
            ones1_sb = small.tile([128, 128], FP32, tag="ones1")
            nc.vector.memset(ones1_sb, 1.0)

            pcs = small.tile([128, 8], FP32, tag="pcs")        # (s,t): s*4+t
            stats128 = small.tile([128, 8], FP32, tag="st128")  # (j,t): j*4+t
            a_pc = small.tile([128, CT], FP32, tag="a_pc")
            beff = small.tile([128, CT], FP32, tag="beff")
            qbias = small.tile([128, CT], FP32, tag="qbias")
            kbias = small.tile([128, CT], FP32, tag="kbias")
            vbias = small.tile([128, CT], FP32, tag="vbias")
            boeff = small.tile([128, CT], FP32, tag="boeff")

            with tc.tile_pool(name="wkvmm", bufs=1) as wkvmm:
                wk_mm = wkvmm.tile([128, CT, C], DT, tag="wkm")
                wv_mm = wkvmm.tile([128, CT, C], DT, tag="wvm")

                with tc.tile_pool(name="wraw", bufs=1) as wraw:
                    wq_sb = wraw.tile([128, CT, C], FP32, tag="wq")
                    wk_sb = wraw.tile([128, CT, C], FP32, tag="wk")
                    wv_sb = wraw.tile([128, CT, C], FP32, tag="wv")
                    wo_sb = wraw.tile([128, CT, C], FP32, tag="wo")

                    # ============ phase 1: GroupNorm statistics =============
                    with (
                        tc.tile_pool(name="xstat", bufs=2) as xstat,
                        tc.tile_pool(name="sttmp", bufs=4) as sttmp,
                    ):
                        for ct in range(CT):
                            xt = xstat.tile([128, N], FP32, tag="xt")
                            nc.sync.dma_start(out=xt, in_=x_r[ct])
                            st = sttmp.tile([128, 8, 6], FP32, tag="st")
                            for j in range(8):
                                nc.vector.bn_stats(
                                    out=st[:, j], in_=xt[:, j * 512:(j + 1) * 512]
                                )
                            mv = sttmp.tile([128, 2], FP32, tag="mv")
                            nc.vector.bn_aggr(out=mv, in_=st)
                            # pcs[:, ct]=mean ; pcs[:, 4+ct]=E[x^2]=var+mean^2
                            nc.vector.tensor_copy(pcs[:, ct:ct + 1], mv[:, 0:1])
                            m2 = sttmp.tile([128, 1], FP32, tag="m2")
                            nc.vector.tensor_mul(m2, mv[:, 0:1], mv[:, 0:1])
                            nc.vector.tensor_add(
                                pcs[:, 4 + ct:5 + ct], mv[:, 1:2], m2
                            )

                    # weight loads after the stats x-loads: stats are
                    # the serial head, weights only gate phase 2
                    for t, d in ((wk_sb, wkT), (wq_sb, wqT),
                                 (wv_sb, wvT), (wo_sb, woT)):
                        nc.sync.dma_start(
                            out=t, in_=d[:].rearrange("(t p) d -> p t d", p=128)
                        )

                    # group sums over the 64 member channels' stats
                    gs_ps = ps_mm.tile([128, 512], FP32, tag="mm")
                    nc.tensor.matmul(
                        gs_ps[:2, :8], lhsT=ind128_sb, rhs=pcs, start=True, stop=True
                    )
                    gs_sb = small.tile([128, 8], FP32, tag="gs")
                    nc.scalar.activation(
                        gs_sb[:2], gs_ps[:2, :8], AF.Copy, scale=1.0 / (C // G)
                    )
                    nc.vector.memset(stats128, 0.0)
                    nc.vector.tensor_copy(stats128[:2, 0:4], gs_sb[:2, 0:4])
                    vtmp = small.tile([128, 4], FP32, tag="vtmp")
                    nc.vector.tensor_mul(vtmp[:2], gs_sb[:2, 0:4], gs_sb[:2, 0:4])
                    nc.vector.tensor_sub(
                        stats128[:2, 4:8], gs_sb[:2, 4:8], vtmp[:2]
                    )
                    nc.scalar.activation(
                        stats128[:2, 4:8], stats128[:2, 4:8], AF.Sqrt,
                        bias=eps_sb[:2],
                    )
                    nc.vector.reciprocal(stats128[:2, 4:8], stats128[:2, 4:8])

                    # broadcast group stats back to channels: bc[p, (j,t)]
                    bc_ps = ps_mm.tile([128, 512], FP32, tag="mm")
                    nc.tensor.matmul(
                        bc_ps[:, :8], lhsT=indT2_sb, rhs=stats128,
                        start=True, stop=True,
                    )
                    bc_sb = small.tile([128, 8], FP32, tag="bc")
                    nc.scalar.copy(bc_sb, bc_ps[:, :8])
                    # a = rstd * gn_w ; beff = gn_b - mean * a
                    nc.vector.tensor_mul(a_pc, bc_sb[:, 4:8], gnw_sb)
                    nc.vector.tensor_mul(beff, bc_sb[:, 0:4], a_pc)
                    nc.vector.tensor_sub(beff, gnb_sb, beff)

                    # ====== phase 2: fold GN into biases and weights ========
                    # qbias[d] = bq[d] + sum_c wqT[c, d] * beff[c]   (etc.)
                    for w_sb, b_sb, out_t in (
                        (wq_sb, bq_sb, qbias),
                        (wk_sb, bk_sb, kbias),
                        (wv_sb, bv_sb, vbias),
                    ):
                        b_ps = ps_mm.tile([128, 512], FP32, tag="mm")
                        for dt in range(CT):
                            for ct in range(CT):
                                nc.tensor.matmul(
                                    b_ps[:, dt:dt + 1],
                                    lhsT=w_sb[:, ct, dt * 128:(dt + 1) * 128],
                                    rhs=beff[:, ct:ct + 1],
                                    start=(ct == 0),
                                    stop=(ct == CT - 1),
                                )
                        nc.vector.tensor_add(out_t, b_ps[:, 0:CT], b_sb)

                    # boeff[e] = bo[e] + sum_d woT[d, e] * vbias[d]
                    bo_ps = ps_mm.tile([128, 512], FP32, tag="mm")
                    for et in range(CT):
                        for dt in range(CT):
                            nc.tensor.matmul(
                                bo_ps[:, et:et + 1],
                                lhsT=wo_sb[:, dt, et * 128:(et + 1) * 128],
                                rhs=vbias[:, dt:dt + 1],
                                start=(dt == 0),
                                stop=(dt == CT - 1),
                            )
                    nc.vector.tensor_add(boeff, bo_ps[:, 0:CT], bo_sb)

                    if DEBUG_DUMP:
                        nc.sync.dma_start(out=dbg["dbg_stats"][:], in_=stats128)
                        nc.sync.dma_start(out=dbg["dbg_a"][:], in_=a_pc)
                        nc.sync.dma_start(out=dbg["dbg_qb"][:], in_=qbias)
                        nc.sync.dma_start(out=dbg["dbg_bo"][:], in_=boeff)

                    # fold a[c] into wq/wk/wv rows (rounding to DT on write);
                    # wo is just rounded
                    for w_sb, w_m in (
                        (wk_sb, wk_mm), (wq_sb, wq_mm), (wv_sb, wv_mm)
                    ):
                        for ct in range(CT):
                            nc.vector.tensor_scalar_mul(
                                w_m[:, ct, :], w_sb[:, ct, :], a_pc[:, ct:ct + 1]
                            )
                    nc.vector.tensor_copy(wo_mm, wo_sb)

                # ========== phase 3: K' [c, m] and V_tok [m, d] =============
                with tc.tile_pool(name="kv", bufs=1) as kvp:
                    k_full = kvp.tile([128, CT, N], DT, tag="k_full")
                    v_full = kvp.tile([128, MC, 512], DT, tag="v_full")

                    with (
                        tc.tile_pool(name="xq", bufs=4) as xq,
                        tc.tile_pool(name="qp", bufs=4) as qpool,
                    ):
                        def emit_qproj(nb):
                            """x cast-load + Q' projection for block nb;
                            emitted one block ahead so the matmuls fill the
                            PE while the denom chain of the previous block
                            runs on DVE/ACT."""
                            nsl_q = slice(nb * NBS, (nb + 1) * NBS)
                            xqs = []
                            for ct in range(CT):
                                xtq = xq.tile([128, NBS], DT, tag="xq",
                                              name=f"xq{nb}_{ct}")
                                dma_cast(xtq, x_r[ct][:, nsl_q])
                                xqs.append(xtq)
                            qs = []
                            for dt in range(CT):
                                qp_ps = ps_mm.tile([128, 512], FP32, tag="mm",
                                                   name=f"qps{nb}_{dt}")
                                for ct in range(CT):
                                    nc.tensor.matmul(
                                        qp_ps,
                                        lhsT=wq_mm[:, ct, dt * 128:(dt + 1) * 128],
                                        rhs=xqs[ct],
                                        start=(ct == 0),
                                        stop=(ct == CT - 1),
                                    )
                                qt = qpool.tile([128, NBS], DT, tag="q",
                                                name=f"q{nb}_{dt}")
                                nc.vector.tensor_scalar_add(
                                    qt, qp_ps, qbias[:, dt:dt + 1]
                                )
                                qs.append(qt)
                            return qs

                        qs_cur = emit_qproj(0)

                        with tc.tile_pool(name="xmc", bufs=8) as xmc:
                            for m2 in range(8):
                                sl = slice(m2 * 512, (m2 + 1) * 512)
                                xts = []
                                for ct in range(CT):
                                    xt = xmc.tile([128, 512], DT, tag="xmc")
                                    dma_cast(xt, x_r[ct][:, sl])
                                    xts.append(xt)
                                for dt in range(CT):
                                    kp = ps_mm.tile([128, 512], FP32, tag="mm")
                                    for ct in range(CT):
                                        nc.tensor.matmul(
                                            kp,
                                            lhsT=wk_mm[:, ct, dt * 128:(dt + 1) * 128],
                                            rhs=xts[ct],
                                            start=(ct == 0),
                                            stop=(ct == CT - 1),
                                        )
                                    nc.vector.tensor_scalar_add(
                                        k_full[:, dt, sl], kp, kbias[:, dt:dt + 1]
                                    )
                                for mt in range(4):
                                    vp = ps_mm.tile([128, 512], FP32, tag="mm")
                                    for ct in range(CT):
                                        nc.tensor.matmul(
                                            vp,
                                            lhsT=xts[ct][:, mt * 128:(mt + 1) * 128],
                                            rhs=wv_mm[:, ct, :],
                                            start=(ct == 0),
                                            stop=(ct == CT - 1),
                                        )
                                    nc.scalar.copy(v_full[:, m2 * 4 + mt, :], vp)

                        if DEBUG_DUMP:
                            nc.sync.dma_start(out=dbg["dbg_k"][:], in_=k_full[:, 0, 0:512])
                            nc.sync.dma_start(out=dbg["dbg_v"][:], in_=v_full[:, 0, :])

                        # ========== phase 4: attention per query block ======
                        with (
                            tc.tile_pool(name="xres", bufs=4) as xres,
                            tc.tile_pool(name="pp", bufs=2) as ppool,
                            tc.tile_pool(name="dn", bufs=2) as dnpool,
                            tc.tile_pool(name="op", bufs=4) as opool,
                            tc.tile_pool(name="yp", bufs=2) as ypool,
                            tc.tile_pool(name="ps_S", bufs=2, space="PSUM") as ps_s,
                            tc.tile_pool(name="ps_O", bufs=4, space="PSUM") as ps_o,
                        ):
                            for nb in range(NB):
                                nsl = slice(nb * NBS, (nb + 1) * NBS)
                                xrs = []
                                for ct in range(CT):
                                    xtr = xres.tile([128, NBS], FP32, tag="xres")
                                    nc.sync.dma_start(out=xtr, in_=x_r[ct][:, nsl])
                                    xrs.append(xtr)
                                qs = qs_cur

                                dn = dnpool.tile([128, NBS], FP32, tag="dn")
                                nc.vector.memset(dn, 0.0)
                                o_ps = [
                                    ps_o.tile([128, 512], FP32, tag="o",
                                              name=f"o_ps{dt}")
                                    for dt in range(CT)
                                ]
                                for mc in range(MC):
                                    sp = ps_s.tile([128, 512], FP32, tag="s")
                                    for dt in range(CT):
                                        nc.tensor.matmul(
                                            sp,
                                            lhsT=k_full[:, dt, mc * 128:(mc + 1) * 128],
                                            rhs=qs[dt],
                                            start=(dt == 0),
                                            stop=(dt == CT - 1),
                                        )
                                    pb = ppool.tile([128, NBS], DT, tag="p")
                                    nc.scalar.activation(
                                        pb, sp, AF.Exp, scale=float(SCALE)
                                    )
                                    if DEBUG_DUMP and nb == 0 and mc == 0:
                                        nc.sync.dma_start(out=dbg["dbg_p"][:], in_=pb)
                                    nc.vector.tensor_add(dn, dn, pb)
                                    for dt in range(CT):
                                        nc.tensor.matmul(
                                            o_ps[dt],
                                            lhsT=v_full[:, mc, dt * 128:(dt + 1) * 128],
                                            rhs=pb,
                                            start=(mc == 0),
                                            stop=(mc == MC - 1),
                                        )

                                # O evictions (unscaled) go to ACT right away
                                os_ = []
                                for dt in range(CT):
                                    ot = opool.tile([128, NBS], DT, tag="ot")
                                    nc.scalar.copy(ot, o_ps[dt])
                                    os_.append(ot)

                                # next block's Q fills the PE while the denom
                                # chain completes on DVE/ACT
                                qs_cur = emit_qproj(nb + 1) if nb + 1 < NB else None

                                # denom = sum_m P via ones-matmul; reciprocal;
                                # broadcast back via a K=1 matmul
                                dn_ps = ps_s.tile([128, 512], FP32, tag="s",
                                                  name="dn_ps")
                                nc.tensor.matmul(
                                    dn_ps[:1, :], lhsT=ones128_sb, rhs=dn,
                                    start=True, stop=True,
                                )
                                r1 = dnpool.tile([128, NBS], FP32, tag="dn",
                                                 name="r1")
                                nc.vector.reciprocal(r1[:1], dn_ps[:1, :])
                                rb_ps = ps_s.tile([128, 512], FP32, tag="s",
                                                  name="rb_ps")
                                nc.tensor.matmul(
                                    rb_ps, lhsT=ones1_sb[:1], rhs=r1[:1],
                                    start=True, stop=True,
                                )
                                rb = dnpool.tile([128, NBS], FP32, tag="dn",
                                                 name="rb")
                                nc.scalar.copy(rb, rb_ps)
                                if DEBUG_DUMP and nb == 0:
                                    nc.sync.dma_start(out=dbg["dbg_dn"][:], in_=dn)
                                    nc.sync.dma_start(out=dbg["dbg_rb"][:], in_=rb)

                                for et in range(CT):
                                    op_ps = ps_o.tile([128, 512], FP32, tag="o",
                                                      name=f"op_ps{et}")
                                    for dt in range(CT):
                                        nc.tensor.matmul(
                                            op_ps,
                                            lhsT=wo_mm[:, dt, et * 128:(et + 1) * 128],
                                            rhs=os_[dt],
                                            start=(dt == 0),
                                            stop=(dt == CT - 1),
                                        )
                                    yt = ypool.tile([128, NBS], FP32, tag="y")
                                    # y = OP*rb + boeff + x
                                    nc.vector.tensor_tensor(
                                        yt, op_ps, rb, op=ALU.mult
                                    )
                                    nc.vector.scalar_tensor_tensor(
                                        yt,
                                        yt,
                                        boeff[:, et:et + 1],
                                        xrs[et],
                                        op0=ALU.add,
                                        op1=ALU.add,
                                    )
                                    nc.sync.dma_start(out=y_r[et][:, nsl], in_=yt)
    if os.environ.get("ATTN_NO_SPLIT", "0") != "1":
        _split_multi_waits(nc)
    return nc


_NC_CACHE = {}


def _get_nc():
    key = (MM_F32R, DEBUG_DUMP)
    if key not in _NC_CACHE:
        _NC_CACHE[key] = _build_kernel()
    return _NC_CACHE[key]


def _make_in_maps(x, gn_w, gn_b, wq, bq, wk, bk, wv, bv, wo, bo):
    x = np.asarray(x, np.float32).reshape(B, C, N)
    shared = {
        "wqT": np.ascontiguousarray(np.asarray(wq, np.float32).T),
        "wkT": np.ascontiguousarray(np.asarray(wk, np.float32).T),
        "wvT": np.ascontiguousarray(np.asarray(wv, np.float32).T),
        "woT": np.ascontiguousarray(np.asarray(wo, np.float32).T),
        "gnw": np.asarray(gn_w, np.float32),
        "gnb": np.asarray(gn_b, np.float32),
        "bq": np.asarray(bq, np.float32),
        "bk": np.asarray(bk, np.float32),
        "bv": np.asarray(bv, np.float32),
        "bo": np.asarray(bo, np.float32),
    }
    ind128 = np.zeros((128, 2), np.float32)
    ind128[:64, 0] = 1.0
    ind128[64:, 1] = 1.0
    indT2 = np.zeros((128, 128), np.float32)
    indT2[0, :64] = 1.0
    indT2[1, 64:] = 1.0
    shared["ind128"] = ind128
    shared["indT2"] = indT2
    return [
        {"x": np.ascontiguousarray(x[b]), **shared} for b in range(B)
    ]


def run(inputs, trace=False, tmpdir=None):
    nc = _get_nc()
    in_maps = _make_in_maps(**inputs)
    res = run_bass_kernel_spmd(
        nc, in_maps, core_ids=list(range(B)), trace=trace, tmpdir=tmpdir
    )
    out = np.stack([res.results[b]["y"] for b in range(B)])
    return out.reshape(B, C, 64, 64).astype(np.float32), res


def kernel(**inputs):
    out, _ = run(inputs)
    return out


# revision 22
# speedup vs baseline: 1.0616x; 1.0125x over previous
"""Trainium2 Bass kernel for an AttentionBlock (GroupNorm + single-head
self-attention + residual) over x[8, 512, 64, 64].

Sharding: data-parallel over batch — one batch element per NeuronCore (8 cores).
Per-core layout is channel-major [C=512, N=H*W=4096]; attention runs
flash-style over 512-token query blocks with scores kept transposed
[key, query] so no transposes are ever needed:

  GroupNorm is folded into the QKV weights (w*a[c]) and biases, so the
  normalized activations are never materialized.  K' [c,m] and V_tok [m,d]
  are computed once and kept in SBUF; per query block, S^T = K'^T Q' is
  accumulated in PSUM, exponentiated on the scalar engine (no max-subtraction:
  scores are ~N(0,1), exp is safe in fp32), the softmax denominator is
  accumulated on the vector engine and reduced across partitions with a
  ones-matmul, and P@V accumulates into 4 PSUM banks.  The 1/denom scale,
  output projection bias and residual are folded into the evictions.

Matmul dtype: float32r (full-rate PE, 11-bit-mantissa RNE inputs, fp32
accumulate) by default; ATTN_MM_F32R=0 switches to exact fp32 (1/4-rate).
All f32r operands are produced rounded (engine writes to f32r tiles, or
gpsimd casting DMAs) — walrus' BIR verifier requires it, and bitcast views
crash the exec unit.  The residual path keeps an unrounded fp32 copy of x.
"""

import os

import numpy as np

import concourse.bass as bass
import concourse.mybir as mybir
import concourse.tile as tile

from concourse.bass_utils import run_bass_kernel_spmd
from concourse.vector_clock import ScopedClock

AF = mybir.ActivationFunctionType
ALU = mybir.AluOpType
FP32 = mybir.dt.float32
F32R = mybir.dt.float32r

B = 8
C = 512
N = 4096          # H*W
G = 8             # groups
EPS = 1e-5
CT = C // 128     # 4 channel tiles
NBS = 512         # query-block size
NB = N // NBS     # 8 query blocks
MC = N // 128     # 32 key chunks
SCALE = 1.0 / np.sqrt(np.float32(C))

MM_F32R = os.environ.get("ATTN_MM_F32R", "1") == "1"
DEBUG_DUMP = os.environ.get("ATTN_DEBUG_DUMP", "0") == "1"


class _TileContext(tile.TileContext):
    """This container's walrus rejects >1 sync wait on a CTRL instruction
    ("Too many sync wait commands"); split the tail drain's waits across
    multiple drain instructions.  It also rejects long semaphore-range-clear
    ISA instructions ("ISA wrong length"); clear in chunks of <=3."""

    def _drain_and_barrier(self, tick_clock, wait_clock):
        drain_inst = self.nc.sync.drain()
        wait_clock.add_sem_waits(
            drain_inst.ins, ScopedClock({None: tick_clock.global_clock})
        )
        si = drain_inst.ins.sync_info
        if si is not None and si.on_wait and len(si.on_wait) > 1:
            waits = list(si.on_wait)
            drain_inst.ins.sync_info = mybir.SyncInfo(
                on_wait=[waits[0]], on_update=list(si.on_update)
            )
            for w in waits[1:]:
                d = self.nc.sync.drain()
                d.ins.sync_info = mybir.SyncInfo(on_wait=[w], on_update=[])

        self.nc.all_engine_barrier()
        assert self.sems is not None
        popped = self.nc._tile_sem_poison_stack.pop()
        assert popped is self._sem_poison
        sems = list(self.sems.allocated().values())
        for i in range(0, len(sems), 3):
            self.nc.clear_and_free_semaphores(sems[i:i + 3])
        self.nc.all_engine_barrier()


def _split_multi_waits(nc, limit=1):
    """This container's walrus accepts at most one sync wait per instruction.
    Hoist extra waits onto same-engine EventSemaphore instructions inserted
    just before — equivalent ordering (engines execute in program order)."""
    nid = 0
    for f in nc.m.functions:
        for bb in f.blocks:
            out = []
            changed = False
            for inst in bb.instructions:
                si = inst.sync_info
                if si is not None and si.on_wait and len(si.on_wait) > limit:
                    waits = list(si.on_wait)
                    for w in waits[:-limit]:
                        ev = mybir.InstEventSemaphore(
                            name=f"I-wsplit-{nid}",
                            engine=inst.engine,
                            sync_info=mybir.SyncInfo(on_wait=[w], on_update=[]),
                        )
                        nid += 1
                        out.append(ev)
                    inst.sync_info = mybir.SyncInfo(
                        on_wait=waits[-limit:], on_update=list(si.on_update)
                    )
                    changed = True
                out.append(inst)
            if changed:
                bb.instructions = out


def _build_kernel():
    DT = F32R if MM_F32R else FP32
    nc = bass.Bass()

    x = nc.declare_dram_parameter("x", [C, N], FP32, isOutput=False)
    wqT = nc.declare_dram_parameter("wqT", [C, C], FP32, isOutput=False)
    wkT = nc.declare_dram_parameter("wkT", [C, C], FP32, isOutput=False)
    wvT = nc.declare_dram_parameter("wvT", [C, C], FP32, isOutput=False)
    woT = nc.declare_dram_parameter("woT", [C, C], FP32, isOutput=False)
    gnw = nc.declare_dram_parameter("gnw", [C], FP32, isOutput=False)
    gnb = nc.declare_dram_parameter("gnb", [C], FP32, isOutput=False)
    bq = nc.declare_dram_parameter("bq", [C], FP32, isOutput=False)
    bk = nc.declare_dram_parameter("bk", [C], FP32, isOutput=False)
    bv = nc.declare_dram_parameter("bv", [C], FP32, isOutput=False)
    bo = nc.declare_dram_parameter("bo", [C], FP32, isOutput=False)
    # group-indicator constants for the cross-partition GroupNorm reductions
    ind128 = nc.declare_dram_parameter("ind128", [128, 2], FP32, isOutput=False)
    indT2 = nc.declare_dram_parameter("indT2", [128, 128], FP32, isOutput=False)
    y = nc.declare_dram_parameter("y", [C, N], FP32, isOutput=True)
    dbg = {}
    if DEBUG_DUMP:
        for nm, shp in [
            ("dbg_stats", [128, 8]), ("dbg_a", [128, CT]),
            ("dbg_qb", [128, CT]), ("dbg_bo", [128, CT]),
            ("dbg_q", [128, 512]), ("dbg_k", [128, 512]),
            ("dbg_v", [128, 512]), ("dbg_p", [128, 512]),
            ("dbg_dn", [128, 512]), ("dbg_rb", [128, 512]),
        ]:
            dbg[nm] = nc.declare_dram_parameter(nm, shp, FP32, isOutput=True)

    x_r = x[:].rearrange("(t p) m -> t p m", p=128)   # [4, 128, 4096]
    y_r = y[:].rearrange("(t p) m -> t p m", p=128)

    def dma_cast(out, in_):
        # only gpsimd DMAs may cast fp32 -> f32r
        if out.dtype != in_.dtype:
            nc.gpsimd.dma_start(out=out, in_=in_)
        else:
            nc.sync.dma_start(out=out, in_=in_)

    with _TileContext(nc) as tc:
        with (
            tc.tile_pool(name="small", bufs=1) as small,
            tc.tile_pool(name="wmm", bufs=1) as wmm,
            tc.tile_pool(name="ps_mm", bufs=2, space="PSUM") as ps_mm,
        ):
            # ---- persistent: folded/rounded weights used in phase 4 ----
            wq_mm = wmm.tile([128, CT, C], DT, tag="wqm")
            wo_mm = wmm.tile([128, CT, C], DT, tag="wom")

            ind128_sb = small.tile([128, 2], FP32, tag="ind128")
            indT2_sb = small.tile([128, 128], FP32, tag="indT2")
            nc.sync.dma_start(out=ind128_sb, in_=ind128[:])
            nc.sync.dma_start(out=indT2_sb, in_=indT2[:])

            def load_pc(name, dram):  # [512] -> [128, 4] (channel = t*128+p)
                t = small.tile([128, CT], FP32, tag=name)
                nc.sync.dma_start(out=t, in_=dram[:].rearrange("(t p) -> p t", p=128))
                return t

            gnw_sb = load_pc("gnw", gnw)
            gnb_sb = load_pc("gnb", gnb)
            bq_sb = load_pc("bq", bq)
            bk_sb = load_pc("bk", bk)
            bv_sb = load_pc("bv", bv)
            bo_sb = load_pc("bo", bo)

            eps_sb = small.tile([128, 1], FP32, tag="eps")
            nc.vector.memset(eps_sb, EPS)
            ones128_sb = small.tile([128, 1], FP32, tag="ones128")
            nc.vector.memset(ones128_sb, 1.0)
            ones1_sb = small.tile([128, 128], FP32, tag="ones1")
            nc.vector.memset(ones1_sb, 1.0)

            pcs = small.tile([128, 8], FP32, tag="pcs")        # (s,t): s*4+t
            stats128 = small.tile([128, 8], FP32, tag="st128")  # (j,t): j*4+t
            a_pc = small.tile([128, CT], FP32, tag="a_pc")
            beff = small.tile([128, CT], FP32, tag="beff")
            qbias = small.tile([128, CT], FP32, tag="qbias")
            kbias = small.tile([128, CT], FP32, tag="kbias")
            vbias = small.tile([128, CT], FP32, tag="vbias")
            boeff = small.tile([128, CT], FP32, tag="boeff")

            with tc.tile_pool(name="wkvmm", bufs=1) as wkvmm:
                wk_mm = wkvmm.tile([128, CT, C], DT, tag="wkm")
                wv_mm = wkvmm.tile([128, CT, C], DT, tag="wvm")

                with tc.tile_pool(name="wraw", bufs=1) as wraw:
                    wq_sb = wraw.tile([128, CT, C], FP32, tag="wq")
                    wk_sb = wraw.tile([128, CT, C], FP32, tag="wk")
                    wv_sb = wraw.tile([128, CT, C], FP32, tag="wv")
                    wo_sb = wraw.tile([128, CT, C], FP32, tag="wo")

                    # ============ phase 1: GroupNorm statistics =============
                    with (
                        tc.tile_pool(name="xstat", bufs=2) as xstat,
                        tc.tile_pool(name="sttmp", bufs=4) as sttmp,
                    ):
                        for ct in range(CT):
                            xt = xstat.tile([128, N], FP32, tag="xt")
                            # chunked loads so bn_stats overlaps the DMA
                            for h in range(4):
                                hs = slice(h * 1024, (h + 1) * 1024)
                                nc.sync.dma_start(out=xt[:, hs], in_=x_r[ct][:, hs])
                            st = sttmp.tile([128, 8, 6], FP32, tag="st")
                            for j in range(8):
                                nc.vector.bn_stats(
                                    out=st[:, j], in_=xt[:, j * 512:(j + 1) * 512]
                                )
                            mv = sttmp.tile([128, 2], FP32, tag="mv")
                            nc.vector.bn_aggr(out=mv, in_=st)
                            # pcs[:, ct]=mean ; pcs[:, 4+ct]=E[x^2]=var+mean^2
                            nc.vector.tensor_copy(pcs[:, ct:ct + 1], mv[:, 0:1])
                            m2 = sttmp.tile([128, 1], FP32, tag="m2")
                            nc.vector.tensor_mul(m2, mv[:, 0:1], mv[:, 0:1])
                            nc.vector.tensor_add(
                                pcs[:, 4 + ct:5 + ct], mv[:, 1:2], m2
                            )

                    # weight loads after the stats x-loads: stats are
                    # the serial head, weights only gate phase 2
                    for t, d in ((wk_sb, wkT), (wq_sb, wqT),
                                 (wv_sb, wvT), (wo_sb, woT)):
                        nc.sync.dma_start(
                            out=t, in_=d[:].rearrange("(t p) d -> p t d", p=128)
                        )

                    # group sums over the 64 member channels' stats
                    gs_ps = ps_mm.tile([128, 512], FP32, tag="mm")
                    nc.tensor.matmul(
                        gs_ps[:2, :8], lhsT=ind128_sb, rhs=pcs, start=True, stop=True
                    )
                    gs_sb = small.tile([128, 8], FP32, tag="gs")
                    nc.scalar.activation(
                        gs_sb[:2], gs_ps[:2, :8], AF.Copy, scale=1.0 / (C // G)
                    )
                    nc.vector.memset(stats128, 0.0)
                    nc.vector.tensor_copy(stats128[:2, 0:4], gs_sb[:2, 0:4])
                    vtmp = small.tile([128, 4], FP32, tag="vtmp")
                    nc.vector.tensor_mul(vtmp[:2], gs_sb[:2, 0:4], gs_sb[:2, 0:4])
                    nc.vector.tensor_sub(
                        stats128[:2, 4:8], gs_sb[:2, 4:8], vtmp[:2]
                    )
                    nc.scalar.activation(
                        stats128[:2, 4:8], stats128[:2, 4:8], AF.Sqrt,
                        bias=eps_sb[:2],
                    )
                    nc.vector.reciprocal(stats128[:2, 4:8], stats128[:2, 4:8])

                    # broadcast group stats back to channels: bc[p, (j,t)]
                    bc_ps = ps_mm.tile([128, 512], FP32, tag="mm")
                    nc.tensor.matmul(
                        bc_ps[:, :8], lhsT=indT2_sb, rhs=stats128,
                        start=True, stop=True,
                    )
                    bc_sb = small.tile([128, 8], FP32, tag="bc")
                    nc.scalar.copy(bc_sb, bc_ps[:, :8])
                    # a = rstd * gn_w ; beff = gn_b - mean * a
                    nc.vector.tensor_mul(a_pc, bc_sb[:, 4:8], gnw_sb)
                    nc.vector.tensor_mul(beff, bc_sb[:, 0:4], a_pc)
                    nc.vector.tensor_sub(beff, gnb_sb, beff)

                    # ====== phase 2: fold GN into biases and weights ========
                    # qbias[d] = bq[d] + sum_c wqT[c, d] * beff[c]   (etc.)
                    for w_sb, b_sb, out_t in (
                        (wq_sb, bq_sb, qbias),
                        (wk_sb, bk_sb, kbias),
                        (wv_sb, bv_sb, vbias),
                    ):
                        b_ps = ps_mm.tile([128, 512], FP32, tag="mm")
                        for dt in range(CT):
                            for ct in range(CT):
                                nc.tensor.matmul(
                                    b_ps[:, dt:dt + 1],
                                    lhsT=w_sb[:, ct, dt * 128:(dt + 1) * 128],
                                    rhs=beff[:, ct:ct + 1],
                                    start=(ct == 0),
                                    stop=(ct == CT - 1),
                                )
                        nc.vector.tensor_add(out_t, b_ps[:, 0:CT], b_sb)

                    # boeff[e] = bo[e] + sum_d woT[d, e] * vbias[d]
                    bo_ps = ps_mm.tile([128, 512], FP32, tag="mm")
                    for et in range(CT):
                        for dt in range(CT):
                            nc.tensor.matmul(
                                bo_ps[:, et:et + 1],
                                lhsT=wo_sb[:, dt, et * 128:(et + 1) * 128],
                                rhs=vbias[:, dt:dt + 1],
                                start=(dt == 0),
                                stop=(dt == CT - 1),
                            )
                    nc.vector.tensor_add(boeff, bo_ps[:, 0:CT], bo_sb)

                    if DEBUG_DUMP:
                        nc.sync.dma_start(out=dbg["dbg_stats"][:], in_=stats128)
                        nc.sync.dma_start(out=dbg["dbg_a"][:], in_=a_pc)
                        nc.sync.dma_start(out=dbg["dbg_qb"][:], in_=qbias)
                        nc.sync.dma_start(out=dbg["dbg_bo"][:], in_=boeff)

                    # fold a[c] into wq/wk/wv rows (rounding to DT on write);
                    # wo is just rounded
                    for w_sb, w_m in (
                        (wk_sb, wk_mm), (wq_sb, wq_mm), (wv_sb, wv_mm)
                    ):
                        for ct in range(CT):
                            nc.vector.tensor_scalar_mul(
                                w_m[:, ct, :], w_sb[:, ct, :], a_pc[:, ct:ct + 1]
                            )
                    nc.vector.tensor_copy(wo_mm, wo_sb)

                # ========== phase 3: K' [c, m] and V_tok [m, d] =============
                with tc.tile_pool(name="kv", bufs=1) as kvp:
                    k_full = kvp.tile([128, CT, N], DT, tag="k_full")
                    v_full = kvp.tile([128, MC, 512], DT, tag="v_full")

                    with (
                        tc.tile_pool(name="xq", bufs=4) as xq,
                        tc.tile_pool(name="qp", bufs=4) as qpool,
                    ):
                        def emit_qproj(nb):
                            """x cast-load + Q' projection for block nb;
                            emitted one block ahead so the matmuls fill the
                            PE while the denom chain of the previous block
                            runs on DVE/ACT."""
                            nsl_q = slice(nb * NBS, (nb + 1) * NBS)
                            xqs = []
                            for ct in range(CT):
                                xtq = xq.tile([128, NBS], DT, tag="xq",
                                              name=f"xq{nb}_{ct}")
                                dma_cast(xtq, x_r[ct][:, nsl_q])
                                xqs.append(xtq)
                            qs = []
                            for dt in range(CT):
                                qp_ps = ps_mm.tile([128, 512], FP32, tag="mm",
                                                   name=f"qps{nb}_{dt}")
                                for ct in range(CT):
                                    nc.tensor.matmul(
                                        qp_ps,
                                        lhsT=wq_mm[:, ct, dt * 128:(dt + 1) * 128],
                                        rhs=xqs[ct],
                                        start=(ct == 0),
                                        stop=(ct == CT - 1),
                                    )
                                qt = qpool.tile([128, NBS], DT, tag="q",
                                                name=f"q{nb}_{dt}")
                                nc.vector.tensor_scalar_add(
                                    qt, qp_ps, qbias[:, dt:dt + 1]
                                )
                                qs.append(qt)
                            return qs

                        qs_cur = emit_qproj(0)

                        with tc.tile_pool(name="xmc", bufs=8) as xmc:
                            for m2 in range(8):
                                sl = slice(m2 * 512, (m2 + 1) * 512)
                                xts = []
                                for ct in range(CT):
                                    xt = xmc.tile([128, 512], DT, tag="xmc")
                                    dma_cast(xt, x_r[ct][:, sl])
                                    xts.append(xt)
                                for dt in range(CT):
                                    kp = ps_mm.tile([128, 512], FP32, tag="mm")
                                    for ct in range(CT):
                                        nc.tensor.matmul(
                                            kp,
                                            lhsT=wk_mm[:, ct, dt * 128:(dt + 1) * 128],
                                            rhs=xts[ct],
                                            start=(ct == 0),
                                            stop=(ct == CT - 1),
                                        )
                                    nc.vector.tensor_scalar_add(
                                        k_full[:, dt, sl], kp, kbias[:, dt:dt + 1]
                                    )
                                for mt in range(4):
                                    vp = ps_mm.tile([128, 512], FP32, tag="mm")
                                    for ct in range(CT):
                                        nc.tensor.matmul(
                                            vp,
                                            lhsT=xts[ct][:, mt * 128:(mt + 1) * 128],
                                            rhs=wv_mm[:, ct, :],
                                            start=(ct == 0),
                                            stop=(ct == CT - 1),
                                        )
                                    nc.scalar.copy(v_full[:, m2 * 4 + mt, :], vp)

                        if DEBUG_DUMP:
                            nc.sync.dma_start(out=dbg["dbg_k"][:], in_=k_full[:, 0, 0:512])
                            nc.sync.dma_start(out=dbg["dbg_v"][:], in_=v_full[:, 0, :])

                        # ========== phase 4: attention per query block ======
                        with (
                            tc.tile_pool(name="xres", bufs=4) as xres,
                            tc.tile_pool(name="pp", bufs=2) as ppool,
                            tc.tile_pool(name="dn", bufs=2) as dnpool,
                            tc.tile_pool(name="op", bufs=4) as opool,
                            tc.tile_pool(name="yp", bufs=2) as ypool,
                            tc.tile_pool(name="ps_S", bufs=2, space="PSUM") as ps_s,
                            tc.tile_pool(name="ps_O", bufs=4, space="PSUM") as ps_o,
                        ):
                            for nb in range(NB):
                                nsl = slice(nb * NBS, (nb + 1) * NBS)
                                xrs = []
                                for ct in range(CT):
                                    xtr = xres.tile([128, NBS], FP32, tag="xres")
                                    nc.sync.dma_start(out=xtr, in_=x_r[ct][:, nsl])
                                    xrs.append(xtr)
                                qs = qs_cur

                                dn = dnpool.tile([128, NBS], FP32, tag="dn")
                                nc.vector.memset(dn, 0.0)
                                o_ps = [
                                    ps_o.tile([128, 512], FP32, tag="o",
                                              name=f"o_ps{dt}")
                                    for dt in range(CT)
                                ]
                                for mc in range(MC):
                                    sp = ps_s.tile([128, 512], FP32, tag="s")
                                    for dt in range(CT):
                                        nc.tensor.matmul(
                                            sp,
                                            lhsT=k_full[:, dt, mc * 128:(mc + 1) * 128],
                                            rhs=qs[dt],
                                            start=(dt == 0),
                                            stop=(dt == CT - 1),
                                        )
                                    pb = ppool.tile([128, NBS], DT, tag="p")
                                    nc.scalar.activation(
                                        pb, sp, AF.Exp, scale=float(SCALE)
                                    )
                                    if DEBUG_DUMP and nb == 0 and mc == 0:
                                        nc.sync.dma_start(out=dbg["dbg_p"][:], in_=pb)
                                    nc.vector.tensor_add(dn, dn, pb)
                                    for dt in range(CT):
                                        nc.tensor.matmul(
                                            o_ps[dt],
                                            lhsT=v_full[:, mc, dt * 128:(dt + 1) * 128],
                                            rhs=pb,
                                            start=(mc == 0),
                                            stop=(mc == MC - 1),
                                        )

                                # O evictions (unscaled) go to ACT right away
                                os_ = []
                                for dt in range(CT):
                                    ot = opool.tile([128, NBS], DT, tag="ot")
                                    nc.scalar.copy(ot, o_ps[dt])
                                    os_.append(ot)

                                # next block's Q fills the PE while the denom
                                # chain completes on DVE/ACT
                                qs_cur = emit_qproj(nb + 1) if nb + 1 < NB else None

                                # denom = sum_m P via ones-matmul; reciprocal;
                                # broadcast back via a K=1 matmul
                                dn_ps = ps_s.tile([128, 512], FP32, tag="s",
                                                  name="dn_ps")
                                nc.tensor.matmul(
                                    dn_ps[:1, :], lhsT=ones128_sb, rhs=dn,
                                    start=True, stop=True,
                                )
                                r1 = dnpool.tile([128, NBS], FP32, tag="dn",
                                                 name="r1")
                                nc.vector.reciprocal(r1[:1], dn_ps[:1, :])
                                rb_ps = ps_s.tile([128, 512], FP32, tag="s",
                                                  name="rb_ps")
                                nc.tensor.matmul(
                                    rb_ps, lhsT=ones1_sb[:1], rhs=r1[:1],
                                    start=True, stop=True,
                                )
                                rb = dnpool.tile([128, NBS], FP32, tag="dn",
                                                 name="rb")
                                nc.scalar.copy(rb, rb_ps)
                                if DEBUG_DUMP and nb == 0:
                                    nc.sync.dma_start(out=dbg["dbg_dn"][:], in_=dn)
                                    nc.sync.dma_start(out=dbg["dbg_rb"][:], in_=rb)

                                for et in range(CT):
                                    op_ps = ps_o.tile([128, 512], FP32, tag="o",
                                                      name=f"op_ps{et}")
                                    for dt in range(CT):
                                        nc.tensor.matmul(
                                            op_ps,
                                            lhsT=wo_mm[:, dt, et * 128:(et + 1) * 128],
                                            rhs=os_[dt],
                                            start=(dt == 0),
                                            stop=(dt == CT - 1),
                                        )
                                    yt = ypool.tile([128, NBS], FP32, tag="y")
                                    # y = OP*rb + boeff + x
                                    nc.vector.tensor_tensor(
                                        yt, op_ps, rb, op=ALU.mult
                                    )
                                    nc.vector.scalar_tensor_tensor(
                                        yt,
                                        yt,
                                        boeff[:, et:et + 1],
                                        xrs[et],
                                        op0=ALU.add,
                                        op1=ALU.add,
                                    )
                                    nc.sync.dma_start(out=y_r[et][:, nsl], in_=yt)
    if os.environ.get("ATTN_NO_SPLIT", "0") != "1":
        _split_multi_waits(nc)
    return nc


_NC_CACHE = {}


def _get_nc():
    key = (MM_F32R, DEBUG_DUMP)
    if key not in _NC_CACHE:
        _NC_CACHE[key] = _build_kernel()
    return _NC_CACHE[key]


def _make_in_maps(x, gn_w, gn_b, wq, bq, wk, bk, wv, bv, wo, bo):
    x = np.asarray(x, np.float32).reshape(B, C, N)
    shared = {
        "wqT": np.ascontiguousarray(np.asarray(wq, np.float32).T),
        "wkT": np.ascontiguousarray(np.asarray(wk, np.float32).T),
        "wvT": np.ascontiguousarray(np.asarray(wv, np.float32).T),
        "woT": np.ascontiguousarray(np.asarray(wo, np.float32).T),
        "gnw": np.asarray(gn_w, np.float32),
        "gnb": np.asarray(gn_b, np.float32),
        "bq": np.asarray(bq, np.float32),
        "bk": np.asarray(bk, np.float32),
        "bv": np.asarray(bv, np.float32),
        "bo": np.asarray(bo, np.float32),
    }
    ind128 = np.zeros((128, 2), np.float32)
    ind128[:64, 0] = 1.0
    ind128[64:, 1] = 1.0
    indT2 = np.zeros((128, 128), np.float32)
    indT2[0, :64] = 1.0
    indT2[1, 64:] = 1.0
    shared["ind128"] = ind128
    shared["indT2"] = indT2
    return [
        {"x": np.ascontiguousarray(x[b]), **shared} for b in range(B)
    ]


def run(inputs, trace=False, tmpdir=None):
    nc = _get_nc()
    in_maps = _make_in_maps(**inputs)
    res = run_bass_kernel_spmd(
        nc, in_maps, core_ids=list(range(B)), trace=trace, tmpdir=tmpdir
    )
    out = np.stack([res.results[b]["y"] for b in range(B)])
    return out.reshape(B, C, 64, 64).astype(np.float32), res


def kernel(**inputs):
    out, _ = run(inputs)
    return out
